# revision 14
# baseline (speedup 1.0000x reference)
"""Trainium2 Bass kernel for nn_BorderHead: directional-features + 3x3 conv + 1x1 conv.

Algorithm: the module is linear, so fold everything into one 5x5 conv with 80
output channels (w2 folded in) + border-line corrections:
    out = conv5x5(V5, x, pad=2) + 4 border-band line convs + 4 corner terms + bias
The folding maps (9C 3x3 blocks -> 5x5 / band kernels) are extracted numerically
via one-hot weight-basis probes on a tiny C=1 proxy; they are universal constants
of the shift structure, so this is exact (validated to 3e-8 vs reference).

Device: 8 NeuronCores, data-parallel over batch (4 images each). Per image the
5x5 conv runs as 7 spatial tiles (8 rows x 56 cols = 448 free dim) x 50
accumulating bf16 matmuls (2 k-tiles x 25 taps), M=80 output channels, from a
zero-padded [128, 4, 60, 60] bf16 SBUF copy of x. Border corrections are
image-batched matmuls into 4 dedicated PSUM band tiles, merged on DVE.
"""
import numpy as np
import ml_dtypes

SHIFTS = [(1, 0), (-1, 0), (0, 1), (0, -1), (1, 1), (1, -1), (-1, 1), (-1, -1)]
INV_SQRT2 = 1.0 / np.sqrt(2.0)

N_CORES = 8
NIMG = 4           # images per core
C, H, W, O = 256, 56, 56, 80
KT = 2             # 128-channel contraction tiles
HP, WP = 60, 60    # padded spatial
RT = 8             # output rows per spatial tile
NT = H // RT       # 7 spatial tiles per image
BF16 = ml_dtypes.bfloat16

# ---------------------------------------------------------------------------
# host-side: numpy reference pieces for probe extraction
# ---------------------------------------------------------------------------

def _conv2d_np(x, w, pad):
    N, Ci, Hh, Ww = x.shape
    Oo, _, kh, kw = w.shape
    xp = np.zeros((N, Ci, Hh + 2 * pad, Ww + 2 * pad), np.float64)
    xp[:, :, pad:pad + Hh, pad:pad + Ww] = x
    out = np.zeros((N, Oo, Hh, Ww), np.float64)
    for u in range(kh):
        for v in range(kw):
            out += np.einsum("oc,ncij->noij", w[:, :, u, v].astype(np.float64),
                             xp[:, :, u:u + Hh, v:v + Ww].astype(np.float64))
    return out


def _feats_np(x):
    feats = [x]
    for k, (di, dj) in enumerate(SHIFTS):
        d = x - np.roll(x, (di, dj), axis=(2, 3))
        if di == 1: d[:, :, 0, :] = 0.0
        elif di == -1: d[:, :, -1, :] = 0.0
        if dj == 1: d[:, :, :, 0] = 0.0
        elif dj == -1: d[:, :, :, -1] = 0.0
        if k > 3: d = d * INV_SQRT2
        feats.append(d)
    return np.concatenate(feats, axis=1)


def _tiny_out(wbasis, x):
    w1 = wbasis.reshape(1, 9, 3, 3)
    return _conv2d_np(_feats_np(x), w1, pad=1)[0, 0]


def extract_maps(Hp=12, Wp=12):
    """M5 [9,3,3,5,5]; line maps edge->[9,3,3,2,5]; corner maps -> [9,3,3,2,2]."""
    n9 = 81
    M5 = np.zeros((n9, 5, 5))
    Mlines = {e: np.zeros((n9, 2, 5)) for e in ["top", "bot", "left", "right"]}
    Mcorn = {c: np.zeros((n9, 2, 2)) for c in ["tl", "tr", "bl", "br"]}
    i0, j0 = Hp // 2, Wp // 2
    for e in range(n9):
        wb = np.zeros(n9); wb[e] = 1.0; wb = wb.reshape(9, 3, 3)
        x = np.zeros((1, 1, Hp, Wp)); x[0, 0, i0, j0] = 1.0
        out = _tiny_out(wb, x)
        for u in range(5):
            for v in range(5):
                M5[e, u, v] = out[i0 + 2 - u, j0 + 2 - v]

        def resid(x):
            return _tiny_out(wb, x) - _conv2d_np(x, M5[e].reshape(1, 1, 5, 5), pad=2)[0, 0]

        x = np.zeros((1, 1, Hp, Wp)); x[0, 0, 0, j0] = 1.0
        r = resid(x)
        assert np.abs(r[2:, :]).max() < 1e-12
        for rr in range(2):
            for t in range(5):
                Mlines["top"][e, rr, t] = r[rr, j0 - (t - 2)]
        x = np.zeros((1, 1, Hp, Wp)); x[0, 0, Hp - 1, j0] = 1.0
        r = resid(x)
        assert np.abs(r[:Hp - 2, :]).max() < 1e-12
        for rr in range(2):
            for t in range(5):
                Mlines["bot"][e, rr, t] = r[Hp - 1 - rr, j0 - (t - 2)]
        x = np.zeros((1, 1, Hp, Wp)); x[0, 0, i0, 0] = 1.0
        r = resid(x)
        assert np.abs(r[:, 2:]).max() < 1e-12
        for ss in range(2):
            for t in range(5):
                Mlines["left"][e, ss, t] = r[i0 - (t - 2), ss]
        x = np.zeros((1, 1, Hp, Wp)); x[0, 0, i0, Wp - 1] = 1.0
        r = resid(x)
        assert np.abs(r[:, :Wp - 2]).max() < 1e-12
        for ss in range(2):
            for t in range(5):
                Mlines["right"][e, ss, t] = r[i0 - (t - 2), Wp - 1 - ss]

        def corner_resid(ci, cj):
            x = np.zeros((1, 1, Hp, Wp)); x[0, 0, ci, cj] = 1.0
            r = resid(x)
            pred = np.zeros_like(r)
            rows = range(2) if ci == 0 else range(Hp - 1, Hp - 3, -1)
            key = "top" if ci == 0 else "bot"
            for rr, row in enumerate(rows):
                for t in range(5):
                    j = cj - (t - 2)
                    if 0 <= j < Wp: pred[row, j] += Mlines[key][e, rr, t]
            cols = range(2) if cj == 0 else range(Wp - 1, Wp - 3, -1)
            key = "left" if cj == 0 else "right"
            for ss, col in enumerate(cols):
                for t in range(5):
                    i = ci - (t - 2)
                    if 0 <= i < Hp: pred[i, col] += Mlines[key][e, ss, t]
            return r - pred

        rc = corner_resid(0, 0)
        assert np.abs(rc[2:, :]).max() < 1e-12 and np.abs(rc[:, 2:]).max() < 1e-12
        Mcorn["tl"][e] = rc[:2, :2]
        rc = corner_resid(0, Wp - 1)
        assert np.abs(rc[2:, :]).max() < 1e-12 and np.abs(rc[:, :Wp - 2]).max() < 1e-12
        Mcorn["tr"][e] = rc[:2, Wp - 2:][:, ::-1]
        rc = corner_resid(Hp - 1, 0)
        assert np.abs(rc[:Hp - 2, :]).max() < 1e-12 and np.abs(rc[:, 2:]).max() < 1e-12
        Mcorn["bl"][e] = rc[Hp - 2:, :2][::-1, :]
        rc = corner_resid(Hp - 1, Wp - 1)
        assert np.abs(rc[:Hp - 2, :]).max() < 1e-12 and np.abs(rc[:, :Wp - 2]).max() < 1e-12
        Mcorn["br"][e] = rc[Hp - 2:, Wp - 2:][::-1, ::-1]
    return (M5.reshape(9, 3, 3, 5, 5),
            {k: v.reshape(9, 3, 3, 2, 5) for k, v in Mlines.items()},
            {k: v.reshape(9, 3, 3, 2, 2) for k, v in Mcorn.items()})


def build_weights(w1, b1, w2, b2):
    """Fold w1/w2 into device weight arrays. Returns dict of np arrays."""
    M5, Mlines, Mcorn = extract_maps()
    w2s = w2[:, :, 0, 0].astype(np.float64)
    w1r = w1.reshape(C, 9, C, 3, 3).astype(np.float64)
    WV = np.einsum("om,mkcqp->ockqp", w2s, w1r)          # [O, C, 9, 3, 3]
    V5 = np.einsum("ockqp,kqpuv->ocuv", WV, M5)           # [O, C, 5, 5]
    lines = {e: np.einsum("ockqp,kqprt->ocrt", WV, M) for e, M in Mlines.items()}
    corners = {e: np.einsum("ockqp,kqprs->ocrs", WV, M) for e, M in Mcorn.items()}
    bc = (w2s @ b1.astype(np.float64) + b2).astype(np.float32)

    # wmain [128, KT*25*O]: (c, kt, uv, o)
    V5r = V5.reshape(O, KT, 128, 5, 5)
    wmain = V5r.transpose(2, 1, 3, 4, 0).reshape(128, KT * 25 * O)

    # wline [128, KT*4*2*5*O]: (c, kt, edge, r, t, o); bot/right flipped so the
    # device-side band index ascends with the actual output row/col.
    wl = np.zeros((128, KT, 4, 2, 5, O))
    def put_line(eidx, K, flip):
        Kr = K.reshape(O, KT, 128, 2, 5)
        if flip:
            Kr = Kr[:, :, :, ::-1, :]
        wl[:, :, eidx] = Kr.transpose(2, 1, 3, 4, 0)
    put_line(0, lines["top"], False)
    put_line(1, lines["bot"], True)
    put_line(2, lines["left"], False)
    put_line(3, lines["right"], True)
    wline = wl.reshape(128, KT * 4 * 2 * 5 * O)

    # wcorn [128, KT*4*2*2*O]: (c, kt, corner, r, s, o) in psum coords
    wc = np.zeros((128, KT, 4, 2, 2, O))
    def put_corner(cidx, K, flip_r, flip_s):
        Kr = K.reshape(O, KT, 128, 2, 2)
        if flip_r: Kr = Kr[:, :, :, ::-1, :]
        if flip_s: Kr = Kr[:, :, :, :, ::-1]
        wc[:, :, cidx] = Kr.transpose(2, 1, 3, 4, 0)
    put_corner(0, corners["tl"], False, False)
    put_corner(1, corners["tr"], False, True)
    put_corner(2, corners["bl"], True, False)
    put_corner(3, corners["br"], True, True)
    wcorn = wc.reshape(128, KT * 4 * 2 * 2 * O)

    return {
        "wmain": wmain.astype(BF16),
        "wline": wline.astype(BF16),
        "wcorn": wcorn.astype(BF16),
        "bc": bc.reshape(O, 1),
    }


# ---------------------------------------------------------------------------
# device program
# ---------------------------------------------------------------------------

def emit_program(tc, x_d, wmain_d, wline_d, wcorn_d, bc_d, out_d):
    import concourse.mybir as mybir

    nc = tc.nc
    f32 = mybir.dt.float32
    bf16 = mybir.dt.bfloat16
    if True:
        with (
            tc.tile_pool(name="weights", bufs=1) as wpool,
            tc.tile_pool(name="xpad", bufs=1) as xpool,
            tc.tile_pool(name="stage", bufs=3) as spool,
            tc.tile_pool(name="osb", bufs=16) as opool,
            tc.tile_pool(name="mm", bufs=4, space="PSUM") as mpool,
            tc.tile_pool(name="band", bufs=1, space="PSUM") as bpool,
        ):
            wmain = wpool.tile([128, KT * 25 * O], bf16, tag="wmain")
            wline = wpool.tile([128, KT * 4 * 2 * 5 * O], bf16, tag="wline")
            wcorn = wpool.tile([128, KT * 4 * 2 * 2 * O], bf16, tag="wcorn")
            bc_sb = wpool.tile([O, 1], f32, tag="bc")
            nc.sync.dma_start(wmain[:], wmain_d[:])
            nc.sync.dma_start(wline[:], wline_d[:])
            nc.sync.dma_start(wcorn[:], wcorn_d[:])
            nc.sync.dma_start(bc_sb[:], bc_d[:])

            xpad = [xpool.tile([128, NIMG, HP, WP], bf16, tag=f"xp{kt}",
                               name=f"xpad{kt}")
                    for kt in range(KT)]

            # load + pad + cast each image / k-tile
            for n in range(NIMG):
                for kt in range(KT):
                    nc.gpsimd.memset(xpad[kt][:, n], 0.0)
                    stg = spool.tile([128, H, W], f32, tag="stg")
                    nc.sync.dma_start(stg[:], x_d[n, kt * 128:(kt + 1) * 128])
                    nc.vector.tensor_copy(xpad[kt][:, n, 2:2 + H, 2:2 + W], stg[:])

            ptop = bpool.tile([O, NIMG, 2, W], f32, tag="ptop")
            pbot = bpool.tile([O, NIMG, 2, W], f32, tag="pbot")
            pleft = bpool.tile([O, NIMG, 2, H], f32, tag="pleft")
            pright = bpool.tile([O, NIMG, 2, H], f32, tag="pright")

            def wline_ap(kt, e, r, t):
                col = (((kt * 4 + e) * 2 + r) * 5 + t) * O
                return wline[:, col:col + O]

            def wcorn_ap(kt, cidx, r, s):
                col = (((kt * 4 + cidx) * 2 + r) * 2 + s) * O
                return wcorn[:, col:col + O]

            def emit_main(n, t):
                psum = mpool.tile([O, RT, W], f32, tag="psum")
                idx = 0
                for kt in range(KT):
                    for u in range(5):
                        for v in range(5):
                            nc.tensor.matmul(
                                psum[:],
                                wmain[:, (kt * 25 + u * 5 + v) * O:(kt * 25 + u * 5 + v) * O + O],
                                xpad[kt][:, n, RT * t + u:RT * t + u + RT, v:v + W],
                                start=(idx == 0), stop=(idx == 49))
                            idx += 1
                osb = opool.tile([O, RT, W], f32, tag="osb")
                nc.scalar.activation(osb[:], psum[:],
                                     mybir.ActivationFunctionType.Identity,
                                     bias=bc_sb[:])
                return osb

            def emit_bands():
                # build op lists per band group: (out_ap, lhsT, rhs)
                # matmul out APs must flatten to a single free dim, so edge
                # MMs are per-image; corner MMs batch images (strided run).
                groups = []
                # horizontal bands (top, bot)
                for eidx, xrow, psumt, c0, c1 in ((0, 2, ptop, 0, 1), (1, 57, pbot, 2, 3)):
                    g = []
                    for kt in range(KT):
                        for r in range(2):
                            for t in range(5):
                                for n in range(NIMG):
                                    g.append((psumt[:, n, r, :], wline_ap(kt, eidx, r, t),
                                              xpad[kt][:, n, xrow, t:t + W]))
                    for kt in range(KT):
                        for r in range(2):
                            for s in range(2):
                                for n in range(NIMG):
                                    g.append((psumt[:, n, r, s:s + 1],
                                              wcorn_ap(kt, c0, r, s),
                                              xpad[kt][:, n, xrow, 2:3]))
                                    g.append((psumt[:, n, r, 54 + s:55 + s],
                                              wcorn_ap(kt, c1, r, s),
                                              xpad[kt][:, n, xrow, 57:58]))
                    groups.append(g)
                # vertical bands (left, right)
                for eidx, xcol, psumt in ((2, 2, pleft), (3, 57, pright)):
                    g = []
                    for kt in range(KT):
                        for s in range(2):
                            for t in range(5):
                                for n in range(NIMG):
                                    g.append((psumt[:, n, s, :], wline_ap(kt, eidx, s, t),
                                              xpad[kt][:, n, t:t + H, xcol]))
                    groups.append(g)
                for g in groups:
                    for i, (oap, lhs, rhs) in enumerate(g):
                        nc.tensor.matmul(oap, lhs, rhs,
                                         start=(i == 0), stop=(i == len(g) - 1))

            def emit_merge_store(n, t, osb):
                for s in range(2):
                    nc.vector.tensor_add(osb[:, :, s], osb[:, :, s],
                                         pleft[:, n, s, RT * t:RT * t + RT])
                    nc.vector.tensor_add(osb[:, :, 54 + s], osb[:, :, 54 + s],
                                         pright[:, n, s, RT * t:RT * t + RT])
                if t == 0:
                    nc.vector.tensor_add(osb[:, 0:2, :], osb[:, 0:2, :], ptop[:, n])
                if t == NT - 1:
                    nc.vector.tensor_add(osb[:, RT - 2:RT, :], osb[:, RT - 2:RT, :],
                                         pbot[:, n])
                nc.sync.dma_start(out_d[n, :, RT * t:RT * t + RT, :], osb[:])

            pending = []
            for n in range(2):
                for t in range(NT):
                    pending.append((n, t, emit_main(n, t)))
            emit_bands()
            for n, t, osb in pending:
                emit_merge_store(n, t, osb)
            for n in range(2, NIMG):
                for t in range(NT):
                    osb = emit_main(n, t)
                    emit_merge_store(n, t, osb)


def build_program():
    import concourse.mybir as mybir
    import concourse.tile as tile
    from concourse import bacc

    f32 = mybir.dt.float32
    bf16 = mybir.dt.bfloat16
    nc = bacc.Bacc("TRN2", target_bir_lowering=False, debug=False,
                   num_devices=N_CORES)
    x_d = nc.dram_tensor("x", [NIMG, C, H, W], f32, kind="ExternalInput").ap()
    wmain_d = nc.dram_tensor("wmain", [128, KT * 25 * O], bf16, kind="ExternalInput").ap()
    wline_d = nc.dram_tensor("wline", [128, KT * 4 * 2 * 5 * O], bf16, kind="ExternalInput").ap()
    wcorn_d = nc.dram_tensor("wcorn", [128, KT * 4 * 2 * 2 * O], bf16, kind="ExternalInput").ap()
    bc_d = nc.dram_tensor("bc", [O, 1], f32, kind="ExternalInput").ap()
    out_d = nc.dram_tensor("out", [NIMG, O, H, W], f32, kind="ExternalOutput").ap()
    with tile.TileContext(nc) as tc:
        emit_program(tc, x_d, wmain_d, wline_d, wcorn_d, bc_d, out_d)
    nc.compile()
    return nc


# ---------------------------------------------------------------------------
# entry point
# ---------------------------------------------------------------------------

_PROG = None
TRACE = False       # set True (e.g. from test.py) to collect an NTFF profile
LAST_RESULT = None  # BassKernelResults of the most recent run

def kernel(x, w1, b1, w2, b2):
    global _PROG, LAST_RESULT
    from concourse.bass_utils import run_bass_kernel_spmd

    x = np.ascontiguousarray(np.asarray(x), dtype=np.float32)
    w1 = np.asarray(w1, dtype=np.float32)
    b1 = np.asarray(b1, dtype=np.float32)
    w2 = np.asarray(w2, dtype=np.float32)
    b2 = np.asarray(b2, dtype=np.float32)

    wd = build_weights(w1, b1, w2, b2)
    if _PROG is None:
        _PROG = build_program()
    nc = _PROG

    in_maps = []
    for i in range(N_CORES):
        m = {"x": x[i * NIMG:(i + 1) * NIMG]}
        m.update(wd)
        in_maps.append(m)
    res = run_bass_kernel_spmd(nc, in_maps, core_ids=list(range(N_CORES)),
                               trace=TRACE)
    LAST_RESULT = res
    out = np.concatenate([res.results[i]["out"] for i in range(N_CORES)], axis=0)
    return out.astype(np.float32)


if __name__ == "__main__":
    # smoke test with random data (no reference comparison)
    rng = np.random.default_rng(0)
    x = rng.standard_normal((32, 256, 56, 56)).astype(np.float32)
    w1 = rng.standard_normal((256, 2304, 3, 3)).astype(np.float32) * 0.01
    b1 = np.zeros(256, np.float32)
    w2 = rng.standard_normal((80, 256, 1, 1)).astype(np.float32) * 0.001
    b2 = np.zeros(80, np.float32)
    o = kernel(x, w1, b1, w2, b2)
    print("out", o.shape, o.dtype, float(np.abs(o).mean()))
